# revision 1
# baseline (speedup 1.0000x reference)
"""GCN 2-layer + link decode on 8 TRN2 NeuronCores (full inputs in/out).

Design (dest-sharded, scatter-free):
- Aggregation commutes with the weight matmul: h = relu(segsum(w1*x[src]) @ W1);
  decode tables [u|v] = segsum2(w2*h[src]) @ (W2 @ [WlinA.T|WlinB.T]).
- Each core owns 12544 destination slots. Edge streams sorted by
  (src int16-range, dest chunk, dest); bulk-gathered via dma_gather (int16
  local indices per 32768-row range); routed+weighted into per-chunk PSUM by
  a selection-matrix matmul whose lhsT is built in ONE fused DVE op:
  sel[e,m] = (iota[e,m] == rel_slot[e]) * w[e].
- Per-(chunk,range) cell sizes are max-padded across cores so the schedule is
  SPMD-uniform; per-core routing differences live in the rel/w data.
- h (100352x128 f32) is AllGather'd between layers; uv (4 f32/node padded to
  64 cols for the 256B dma_gather row constraint) AllGather'd before decode.
- Decode: pairs sorted by pos0, sharded by slot windows; u and v streams both
  sel-routed into pair-chunk PSUM; host unshuffles the output rows.
"""
import numpy as np

P = 128
N = 100_000
NSHARD = 12_500
SLOTS = 12_544
CHUNKS = SLOTS // P          # 98
TABROWS = 8 * SLOTS          # 100352
RBOUND = [32768, 65536, 98304]
RLO = [0, 32768, 65536, 98304]
NCORES = 8
CALL_CELLS = 8               # chunks per gather-call window


def _range_of(a):
    return np.searchsorted(RBOUND, a, side="right")


def _wrap_idx(a):
    """[NCORES, T] int16 -> [NCORES, 128, T//16] (16-wrap, 8x replicate)."""
    ncr, t = a.shape
    out = a.reshape(ncr, t // 16, 16).transpose(0, 2, 1)
    return np.ascontiguousarray(np.tile(out, (1, 8, 1)))


def _prep_stream(tab_row, slot, w, nchunks, call_cells):
    """Generic SPMD-uniform stream builder.

    tab_row: [E] global table row per entry; slot: [E] local out slot
    (0..nchunks*128); w: [E] weight; entries already per-core-filtered lists:
    tab_row etc are lists of arrays, one per core.
    Returns static schedule + per-core idx16 / rel / w arrays.
    """
    ncr = len(tab_row)
    # cell = (chunk, range); count per core
    counts = np.zeros((ncr, nchunks, 4), np.int64)
    for c in range(ncr):
        ch = slot[c] // P
        rr = _range_of(tab_row[c])
        np.add.at(counts, (c, ch, rr), 1)
    estar = counts.max(axis=0)                       # [nchunks, 4]

    # layout per range: calls of CALL windows, each padded to 128 multiple
    layout = []
    for r in range(4):
        calls = []
        base = 0
        for k0 in range(0, nchunks, call_cells):
            k1 = min(k0 + call_cells, nchunks)
            cells = estar[k0:k1, r]
            offs = np.concatenate([[0], np.cumsum(cells)]).astype(np.int64)
            n = int(offs[-1])
            n_pad = max(P, ((n + P - 1) // P) * P)
            calls.append(dict(k0=k0, k1=k1, offs=offs, n=n, n_pad=n_pad,
                              base=base))
            base += n_pad
        layout.append(dict(calls=calls, T=base))

    # static schedule: per chunk, matmul descriptors (r, call, blk, sel_col)
    sched = [[] for _ in range(nchunks)]
    selmap = {}
    n_sel = 0
    for r in range(4):
        for ci, call in enumerate(layout[r]["calls"]):
            nblk = call["n_pad"] // P
            offs, k0 = call["offs"], call["k0"]
            for b in range(nblk):
                e0, e1 = b * P, b * P + P
                ks = [k for k in range(call["k0"], call["k1"])
                      if offs[k - k0] < e1 and offs[k - k0 + 1] > e0]
                if not ks:
                    ks = [call["k0"]]
                for k in ks:
                    sched[k].append(dict(r=r, call=ci, blk=b, sel=n_sel))
                    selmap[(r, ci, b, k)] = n_sel
                    n_sel += 1

    idx16 = [np.zeros((ncr, layout[r]["T"]), np.int16) for r in range(4)]
    rel = np.zeros((ncr, P, n_sel), np.float32)
    wgt = np.zeros((ncr, P, n_sel), np.float32)

    for c in range(ncr):
        tr, sl, ww = tab_row[c], slot[c], w[c]
        rr = _range_of(tr)
        ch = sl // P
        # order entries by (range, chunk, slot)
        o = np.lexsort((sl, ch, rr))
        tr, sl, ww, rr, ch = tr[o], sl[o], ww[o], rr[o], ch[o]
        for r in range(4):
            m = rr == r
            if not m.any():
                continue
            trm, slm, wwm, chm = tr[m], sl[m], ww[m], ch[m]
            # position: call base + cell offset + within-cell rank
            cell_cnt = np.zeros(nchunks, np.int64)
            np.add.at(cell_cnt, chm, 1)
            cstart = np.concatenate([[0], np.cumsum(cell_cnt)])
            within = np.arange(len(slm)) - cstart[chm]
            call_id = chm // call_cells
            calls = layout[r]["calls"]
            cbase = np.array([cl["base"] for cl in calls], np.int64)
            # offset of chunk's cell within its call
            cell_off = np.zeros(nchunks, np.int64)
            for ci, cl in enumerate(calls):
                for k in range(cl["k0"], cl["k1"]):
                    cell_off[k] = cl["offs"][k - cl["k0"]]
            pos = cbase[call_id] + cell_off[chm] + within
            idx16[r][c, pos] = (trm - RLO[r]).astype(np.int16)
            # sel column per entry
            relpos = pos - cbase[call_id]
            blk = relpos // P
            pp = relpos % P
            cols = np.array([selmap[(r, int(ci_), int(b_), int(k_))]
                             for ci_, b_, k_ in zip(call_id, blk, chm)],
                            np.int64)
            rel[c, pp, cols] = (slm % P).astype(np.float32)
            wgt[c, pp, cols] = wwm
    return dict(layout=layout, sched=sched, n_sel=n_sel, idx16=idx16,
                rel=rel, wgt=wgt)


def kernel(x, edge_index1, edge_index2, edge_weight1, edge_weight2,
           pos_edge_index, W1, W2, Wlin):
    import concourse.bass as bass
    from concourse import bacc, tile, mybir
    from concourse.bass_utils import run_bass_kernel_spmd
    from concourse.library_config import mlp
    from concourse.masks import make_identity

    f32, i16 = mybir.dt.float32, mybir.dt.int16
    x = np.asarray(x, np.float32)
    W1 = np.asarray(W1, np.float32)
    W2 = np.asarray(W2, np.float32)
    Wlin = np.asarray(Wlin, np.float32)
    e1 = np.asarray(edge_index1).astype(np.int64)
    e2 = np.asarray(edge_index2).astype(np.int64)
    w1 = np.asarray(edge_weight1, np.float32)
    w2 = np.asarray(edge_weight2, np.float32)
    pe = np.asarray(pos_edge_index).astype(np.int64)

    # ---------- host index preprocessing ----------
    x_tab = np.zeros((TABROWS, P), np.float32)
    x_tab[:N] = x
    n2row = (np.arange(N) // NSHARD) * SLOTS + (np.arange(N) % NSHARD)

    def shard_by_dest(src_rows, dst, w):
        owner = dst // NSHARD
        ld = dst - owner * NSHARD
        return ([src_rows[owner == c] for c in range(NCORES)],
                [ld[owner == c] for c in range(NCORES)],
                [w[owner == c] for c in range(NCORES)])

    l1 = _prep_stream(*shard_by_dest(e1[0], e1[1], w1), CHUNKS, CALL_CELLS)
    l2 = _prep_stream(*shard_by_dest(n2row[e2[0]], e2[1], w2),
                      CHUNKS, CALL_CELLS)

    # decode: shard pairs by original index; per core sort by pos0-row and
    # use the local sorted position as the output slot (host unshuffles).
    t0 = n2row[pe[0]]
    t1 = n2row[pe[1]]
    npairs = pe.shape[1]
    pershard = (npairs + NCORES - 1) // NCORES            # 25000
    per_core = ((pershard + P - 1) // P) * P              # 25088
    dchunks = per_core // P
    pair_slot = np.empty(npairs, np.int64)
    u_rows, u_slots, u_w = [], [], []
    v_rows, v_slots, v_w = [], [], []
    for c in range(NCORES):
        p0, p1 = c * pershard, min((c + 1) * pershard, npairs)
        loc = np.argsort(t0[p0:p1], kind="stable")
        sl = np.empty(p1 - p0, np.int64)
        sl[loc] = np.arange(p1 - p0)
        pair_slot[p0:p1] = c * per_core + sl
        ones = np.ones(p1 - p0, np.float32)
        u_rows.append(t0[p0:p1]); u_slots.append(sl); u_w.append(ones)
        v_rows.append(t1[p0:p1]); v_slots.append(sl); v_w.append(ones)
    du = _prep_stream(u_rows, u_slots, u_w, dchunks, 48)
    dv = _prep_stream(v_rows, v_slots, v_w, dchunks, 48)

    idx_arr = {}
    for key, pr in (("l1", l1), ("l2", l2), ("u", du), ("v", dv)):
        for r in range(4):
            idx_arr[(key, r)] = _wrap_idx(pr["idx16"][r])

    # ---------- device program ----------
    nc = bacc.Bacc("TRN2", target_bir_lowering=False, debug=False,
                   num_devices=NCORES, num_swdge_queues=4)

    def din(name, shape, dt=f32):
        return nc.dram_tensor(name, list(shape), dt, kind="ExternalInput").ap()

    xt = din("x_tab", (TABROWS, P))
    w1t = din("W1r", (P, P))
    w2tt = din("W2T", (P, P))
    wcat = din("Wcat", (P, 4))
    iota_in = din("iota", (P, P))
    idx_in = {k: din(f"idx_{k[0]}_{k[1]}", v.shape[1:], i16)
              for k, v in idx_arr.items()}
    relw_in = {key: (din(f"rel_{key}", (P, pr["n_sel"])),
                     din(f"w_{key}", (P, pr["n_sel"])))
               for key, pr in (("l1", l1), ("l2", l2), ("u", du), ("v", dv))}

    out_d = nc.dram_tensor("out_dec", [P, 2 * dchunks], f32,
                           kind="ExternalOutput").ap()
    h_slice = nc.dram_tensor("h_slice", [SLOTS, P], f32)
    h_tab = nc.dram_tensor("h_tab", [TABROWS, P], f32, addr_space="Shared")
    uv_slice = nc.dram_tensor("uv_slice", [SLOTS, 64], f32)
    uv_tab = nc.dram_tensor("uv_tab", [TABROWS, 64], f32, addr_space="Shared")

    qn = [0]

    def next_q():
        qn[0] = (qn[0] + 1) % 4
        return qn[0]

    with tile.TileContext(nc) as tc:
        with (
            tc.tile_pool(name="meta", bufs=1) as mp,
            tc.tile_pool(name="stage", bufs=2) as sgp,
            tc.tile_pool(name="idxp", bufs=1) as ixp,
            tc.tile_pool(name="selp", bufs=4) as selp,
            tc.tile_pool(name="work", bufs=3) as wp,
            tc.tile_pool(name="psA", bufs=2, space="PSUM") as ppA,
            tc.tile_pool(name="psB", bufs=2, space="PSUM") as ppB,
        ):
            nc.gpsimd.load_library(mlp)
            iota_t = mp.tile([P, P], f32, name="iota_t")
            nc.sync.dma_start(iota_t[:], iota_in[:])
            ident = mp.tile([P, P], f32, name="ident")
            make_identity(nc, ident[:])
            w1_sb = mp.tile([P, P], f32, name="w1_sb")
            nc.sync.dma_start(w1_sb[:], w1t[:])
            w2t_sb = mp.tile([P, P], f32, name="w2t_sb")
            nc.sync.dma_start(w2t_sb[:], w2tt[:])
            wcat_sb = mp.tile([P, 4], f32, name="wcat_sb")
            nc.sync.dma_start(wcat_sb[:], wcat[:])
            wu_ps = ppB.tile([P, 4], f32, space="PSUM", name="wu_ps",
                             tag="psB")
            nc.tensor.matmul(wu_ps[:], lhsT=w2t_sb[:], rhs=wcat_sb[:],
                             start=True, stop=True)
            wu_sb = mp.tile([P, 4], f32, name="wu_sb")
            nc.vector.tensor_copy(wu_sb[:], wu_ps[:])

            def sel_build(name, rel_sb, w_sb, col):
                sel = selp.tile([P, P], f32, name=name, tag="sel")
                nc.vector.scalar_tensor_tensor(
                    out=sel[:], in0=iota_t[:],
                    scalar=rel_sb[:, col:col + 1],
                    in1=w_sb[:, col:col + 1].to_broadcast([P, P]),
                    op0=mybir.AluOpType.is_equal,
                    op1=mybir.AluOpType.mult)
                return sel

            def load_relw(key, pr):
                rel_sb = ixp.tile([P, pr["n_sel"]], f32,
                                  name=f"rel_{key}_sb", tag="relt")
                w_sb = ixp.tile([P, pr["n_sel"]], f32,
                                name=f"w_{key}_sb", tag="wt")
                nc.sync.dma_start(rel_sb[:], relw_in[key][0][:])
                nc.sync.dma_start(w_sb[:], relw_in[key][1][:])
                return rel_sb, w_sb

            def run_agg(key, pr, tab_ap, nchunks, elem, consume):
                """Gather + sel-route; call consume(k, psum_tile) per chunk."""
                rel_sb, w_sb = load_relw(key, pr)
                idx_sb = []
                for r in range(4):
                    cols = pr["layout"][r]["T"] // 16
                    it = ixp.tile([P, cols], i16, name=f"ix_{key}_{r}",
                                  tag=f"ix{r}")
                    nc.sync.dma_start(it[:], idx_in[(key, r)][:])
                    idx_sb.append(it)
                stage_tiles = {}

                def ensure_call(r, ci):
                    if (r, ci) in stage_tiles:
                        return stage_tiles[(r, ci)]
                    call = pr["layout"][r]["calls"][ci]
                    npad = call["n_pad"]
                    c0 = call["base"] // 16
                    st = sgp.tile([P, (npad // P) * elem], f32,
                                  name=f"st_{key}_{r}_{ci}", tag=f"stage{r}")
                    nc.gpsimd.dma_gather(
                        st[:].rearrange("p (c e) -> p c e", e=elem),
                        tab_ap[RLO[r]:], idx_sb[r][:, c0:c0 + npad // 16],
                        npad, npad, elem,
                        queue_num=next_q(), single_packet=False)
                    stage_tiles[(r, ci)] = st
                    return st

                for k in range(nchunks):
                    psum_k = ppA.tile([P, elem], f32, space="PSUM",
                                      name=f"ps_{key}_{k}", tag="psA")
                    descs = pr["sched"][k]
                    for j, d in enumerate(descs):
                        st = ensure_call(d["r"], d["call"])
                        sel = sel_build(f"sel_{key}_{k}_{j}", rel_sb, w_sb,
                                        d["sel"])
                        nc.tensor.matmul(
                            psum_k[:], lhsT=sel[:],
                            rhs=st[:, d["blk"] * elem:(d["blk"] + 1) * elem],
                            start=(j == 0), stop=(j == len(descs) - 1))
                    consume(k, psum_k)

            def consume_l1(k, psum_k):
                a_sb = wp.tile([P, P], f32, name=f"a1_{k}", tag="a")
                nc.vector.tensor_copy(a_sb[:], psum_k[:])
                at_ps = ppB.tile([P, P], f32, space="PSUM",
                                 name=f"at1_{k}", tag="psB")
                nc.tensor.transpose(at_ps[:], a_sb[:], ident[:])
                at_sb = wp.tile([P, P], f32, name=f"at1s_{k}", tag="at")
                nc.vector.tensor_copy(at_sb[:], at_ps[:])
                h_ps = ppB.tile([P, P], f32, space="PSUM",
                                name=f"h1_{k}", tag="psB")
                nc.tensor.matmul(h_ps[:], lhsT=at_sb[:], rhs=w1_sb[:],
                                 start=True, stop=True)
                h_sb = wp.tile([P, P], f32, name=f"h1s_{k}", tag="h")
                nc.vector.tensor_scalar_max(h_sb[:], h_ps[:], 0.0)
                nc.sync.dma_start(h_slice[k * P:(k + 1) * P, :], h_sb[:])

            def consume_l2(k, psum_k):
                a_sb = wp.tile([P, P], f32, name=f"a2_{k}", tag="a")
                nc.vector.tensor_copy(a_sb[:], psum_k[:])
                at_ps = ppB.tile([P, P], f32, space="PSUM",
                                 name=f"at2_{k}", tag="psB")
                nc.tensor.transpose(at_ps[:], a_sb[:], ident[:])
                at_sb = wp.tile([P, P], f32, name=f"at2s_{k}", tag="at")
                nc.vector.tensor_copy(at_sb[:], at_ps[:])
                u_ps = ppB.tile([P, 4], f32, space="PSUM",
                                name=f"uv_{k}", tag="psB")
                nc.tensor.matmul(u_ps[:], lhsT=at_sb[:], rhs=wu_sb[:],
                                 start=True, stop=True)
                u_sb = wp.tile([P, 64], f32, name=f"uvs_{k}", tag="u")
                nc.vector.tensor_copy(u_sb[:, 0:4], u_ps[:])
                nc.sync.dma_start(uv_slice[k * P:(k + 1) * P, :], u_sb[:])

            run_agg("l1", l1, xt, CHUNKS, P, consume_l1)
            nc.gpsimd.collective_compute(
                "AllGather", mybir.AluOpType.bypass,
                replica_groups=[list(range(NCORES))],
                ins=[h_slice[:]], outs=[h_tab[:]])
            run_agg("l2", l2, h_tab[:], CHUNKS, P, consume_l2)
            nc.gpsimd.collective_compute(
                "AllGather", mybir.AluOpType.bypass,
                replica_groups=[list(range(NCORES))],
                ins=[uv_slice[:]], outs=[uv_tab[:]])

            # decode: u then v aggregated into [pairs, 64] psums; add
            out_sb = mp.tile([P, 2 * dchunks], f32, name="out_sb")
            u_all = mp.tile([P, 4 * dchunks], f32, name="u_all")

            def consume_u(k, psum_k):
                nc.vector.tensor_copy(u_all[:, 4 * k:4 * k + 4],
                                      psum_k[:, 0:4])

            def consume_v(k, psum_k):
                nc.vector.tensor_tensor(
                    out=out_sb[:, 2 * k:2 * k + 2],
                    in0=u_all[:, 4 * k:4 * k + 2], in1=psum_k[:, 2:4],
                    op=mybir.AluOpType.add)

            run_agg("u", du, uv_tab[:], dchunks, 64, consume_u)
            run_agg("v", dv, uv_tab[:], dchunks, 64, consume_v)
            nc.sync.dma_start(out_d[:], out_sb[:])

    nc.compile()

    # ---------- stage inputs & run ----------
    iota_np = np.broadcast_to(np.arange(P, dtype=np.float32)[None, :],
                              (P, P)).copy()
    wcat_np = np.ascontiguousarray(
        np.concatenate([Wlin[:, :P].T, Wlin[:, P:].T], axis=1))
    in_maps = []
    for c in range(NCORES):
        m = {"x_tab": x_tab, "W1r": W1,
             "W2T": np.ascontiguousarray(W2.T), "Wcat": wcat_np,
             "iota": iota_np}
        for key, pr in (("l1", l1), ("l2", l2), ("u", du), ("v", dv)):
            m[f"rel_{key}"] = np.ascontiguousarray(pr["rel"][c])
            m[f"w_{key}"] = np.ascontiguousarray(pr["wgt"][c])
            for r in range(4):
                m[f"idx_{key}_{r}"] = idx_arr[(key, r)][c]
        in_maps.append(m)

    res = run_bass_kernel_spmd(nc, in_maps, core_ids=list(range(NCORES)),
                               trace=globals().get("TRACE", False))
    globals()["LAST_EXEC_NS"] = res.exec_time_ns

    out = np.zeros((npairs, 2), np.float32)
    for c in range(NCORES):
        o3 = res.results[c]["out_dec"].reshape(P, dchunks, 2)
        m = (pair_slot >= c * per_core) & (pair_slot < (c + 1) * per_core)
        sl = pair_slot[m] - c * per_core
        out[m] = o3[sl % P, sl // P]
    return out



# revision 4
# speedup vs baseline: 1.1856x; 1.1856x over previous
"""GCN 2-layer + link decode on 8 TRN2 NeuronCores (full inputs in/out).

v2 design (dest-sharded, scatter-free, bf16):
- Aggregation commutes with the weight matmul: h = relu(segsum(w1*x[src]) @ W1);
  z = segsum2(w2*h[src]) @ W2.
- All tables/stages/sels in bf16 (fp32 matmul is 2-pass on PE; bf16 is 1).
- Flipped routing matmul: psum[f,d] += stage[e,f].T @ sel[e,d] — the [f,d]
  psum feeds matmul(lhsT=agg[f,d], rhs=W) directly, no transposes.
- sel built in ONE DVE tensor_scalar op: (iota == rel_scalar) * w_scalar,
  per-partition scalars keep the DVE 2x fast path.
- Each core owns 12544 destination slots; edge streams sorted by
  (range, dest chunk, slot); bulk dma_gather with int16 local indices per
  32768-row range; per-(chunk,range) cell sizes max-padded across cores so
  the schedule is SPMD-uniform.
- h (100352x128 bf16) AllGather'd between layers; z stays RESIDENT IN SBUF.
- Decode: pairs sharded by owner(src-endpoint) per side (u-pass by pos0
  owner, v-pass by pos1 owner); SBUF-source transpose dma_gather pulls
  z rows as [f, pairs]; one matmul per 128-pair chunk with Wlin half
  (A for u, B for v) gives [pair, 2] partials; host adds u+v partials.
"""
import numpy as np
import ml_dtypes

BF16 = ml_dtypes.bfloat16
P = 128
N = 100_000
NSHARD = 12_500
SLOTS = 12_544
CHUNKS = SLOTS // P          # 98
TABROWS = 8 * SLOTS          # 100352
RBOUND = [32768, 65536, 98304]
RLO = [0, 32768, 65536, 98304]
NCORES = 8
WINDOW = 12                  # chunks per gather-call window (l1/l2)
DEC_WIN = 4096               # pairs per decode gather window


def _range_of(a):
    return np.searchsorted(RBOUND, a, side="right")


def _wrap_idx(a):
    """[NCORES, T] int16 -> [NCORES, 128, T//16] (16-wrap, 8x replicate)."""
    ncr, t = a.shape
    out = a.reshape(ncr, t // 16, 16).transpose(0, 2, 1)
    return np.ascontiguousarray(np.tile(out, (1, 8, 1)))


def _prep_stream(tab_row, slot, w, nchunks, window):
    """SPMD-uniform gather+route stream builder (dest-major, 4 ranges).

    tab_row/slot/w: per-core lists of arrays (global table row, local out
    slot 0..nchunks*128, weight). Returns static schedule + per-core
    idx16 / rel / w arrays.
    """
    ncr = len(tab_row)
    counts = np.zeros((ncr, nchunks, 4), np.int64)
    for c in range(ncr):
        np.add.at(counts, (c, slot[c] // P, _range_of(tab_row[c])), 1)
    estar = counts.max(axis=0)                       # [nchunks, 4]

    layout = []
    for r in range(4):
        calls = []
        base = 0
        for k0 in range(0, nchunks, window):
            k1 = min(k0 + window, nchunks)
            cells = estar[k0:k1, r]
            offs = np.concatenate([[0], np.cumsum(cells)]).astype(np.int64)
            n = int(offs[-1])
            n_pad = max(P, ((n + P - 1) // P) * P)
            calls.append(dict(k0=k0, k1=k1, offs=offs, n=n, n_pad=n_pad,
                              base=base))
            base += n_pad
        layout.append(dict(calls=calls, T=base))

    sched = [[] for _ in range(nchunks)]
    selmap = {}
    n_sel = 0
    for r in range(4):
        for ci, call in enumerate(layout[r]["calls"]):
            nblk = call["n_pad"] // P
            offs, k0 = call["offs"], call["k0"]
            for b in range(nblk):
                e0, e1 = b * P, b * P + P
                ks = [k for k in range(call["k0"], call["k1"])
                      if offs[k - k0] < e1 and offs[k - k0 + 1] > e0]
                if not ks:
                    ks = [call["k0"]]
                for k in ks:
                    sched[k].append(dict(r=r, call=ci, blk=b, sel=n_sel))
                    selmap[(r, ci, b, k)] = n_sel
                    n_sel += 1

    idx16 = [np.zeros((ncr, layout[r]["T"]), np.int16) for r in range(4)]
    rel = np.zeros((ncr, P, n_sel), np.float32)
    wgt = np.zeros((ncr, P, n_sel), np.float32)

    for c in range(ncr):
        tr, sl, ww = tab_row[c], slot[c], w[c]
        rr = _range_of(tr)
        ch = sl // P
        o = np.lexsort((sl, ch, rr))
        tr, sl, ww, rr, ch = tr[o], sl[o], ww[o], rr[o], ch[o]
        for r in range(4):
            m = rr == r
            if not m.any():
                continue
            trm, slm, wwm, chm = tr[m], sl[m], ww[m], ch[m]
            cell_cnt = np.zeros(nchunks, np.int64)
            np.add.at(cell_cnt, chm, 1)
            cstart = np.concatenate([[0], np.cumsum(cell_cnt)])
            within = np.arange(len(slm)) - cstart[chm]
            call_id = chm // window
            calls = layout[r]["calls"]
            cbase = np.array([cl["base"] for cl in calls], np.int64)
            cell_off = np.zeros(nchunks, np.int64)
            for ci, cl in enumerate(calls):
                for k in range(cl["k0"], cl["k1"]):
                    cell_off[k] = cl["offs"][k - cl["k0"]]
            pos = cbase[call_id] + cell_off[chm] + within
            idx16[r][c, pos] = (trm - RLO[r]).astype(np.int16)
            relpos = pos - cbase[call_id]
            blk = relpos // P
            pp = relpos % P
            cols = np.array([selmap[(r, int(ci_), int(b_), int(k_))]
                             for ci_, b_, k_ in zip(call_id, blk, chm)],
                            np.int64)
            rel[c, pp, cols] = (slm % P).astype(np.float32)
            wgt[c, pp, cols] = wwm
    return dict(layout=layout, sched=sched, n_sel=n_sel, idx16=idx16,
                rel=rel, wgt=wgt)


def kernel(x, edge_index1, edge_index2, edge_weight1, edge_weight2,
           pos_edge_index, W1, W2, Wlin):
    import concourse.bass as bass
    from concourse import bacc, tile, mybir
    from concourse.bass_utils import run_bass_kernel_spmd
    from concourse.library_config import mlp

    f32, bf16, i16 = mybir.dt.float32, mybir.dt.bfloat16, mybir.dt.int16
    AF = mybir.ActivationFunctionType
    x = np.asarray(x, np.float32)
    W1 = np.asarray(W1, np.float32)
    W2 = np.asarray(W2, np.float32)
    Wlin = np.asarray(Wlin, np.float32)
    e1 = np.asarray(edge_index1).astype(np.int64)
    e2 = np.asarray(edge_index2).astype(np.int64)
    w1 = np.asarray(edge_weight1, np.float32)
    w2 = np.asarray(edge_weight2, np.float32)
    pe = np.asarray(pos_edge_index).astype(np.int64)

    # ---------- host index preprocessing ----------
    x_tab = np.zeros((TABROWS, P), BF16)
    x_tab[:N] = x.astype(BF16)
    n2row = (np.arange(N) // NSHARD) * SLOTS + (np.arange(N) % NSHARD)

    def shard_by_dest(src_rows, dst, w):
        owner = dst // NSHARD
        ld = dst - owner * NSHARD
        return ([src_rows[owner == c] for c in range(NCORES)],
                [ld[owner == c] for c in range(NCORES)],
                [w[owner == c] for c in range(NCORES)])

    l1 = _prep_stream(*shard_by_dest(e1[0], e1[1], w1), CHUNKS, WINDOW)
    l2 = _prep_stream(*shard_by_dest(n2row[e2[0]], e2[1], w2),
                      CHUNKS, WINDOW)

    # decode: u-pass sharded by owner(pos0), v-pass by owner(pos1).
    # idx = local z slot of the endpoint; output slot = arrival order.
    npairs = pe.shape[1]

    def shard_pairs(tt):
        owner = tt // NSHARD
        loc = tt - owner * NSHARD
        ids = [np.nonzero(owner == c)[0] for c in range(NCORES)]
        idxs = [loc[i] for i in ids]
        return ids, idxs

    u_ids, u_idx = shard_pairs(pe[0])
    v_ids, v_idx = shard_pairs(pe[1])
    nd_pairs = max(max(len(a) for a in u_idx), max(len(a) for a in v_idx))
    nd_pairs = ((nd_pairs + P - 1) // P) * P
    ndchunks = nd_pairs // P

    def pack_dec_idx(idxs):
        a = np.zeros((NCORES, nd_pairs), np.int16)
        for c in range(NCORES):
            a[c, :len(idxs[c])] = idxs[c].astype(np.int16)
        return _wrap_idx(a)

    u_idx16 = pack_dec_idx(u_idx)
    v_idx16 = pack_dec_idx(v_idx)
    dec_wins = [(w0, min(w0 + DEC_WIN, nd_pairs))
                for w0 in range(0, nd_pairs, DEC_WIN)]

    idx_arr = {}
    for key, pr in (("l1", l1), ("l2", l2)):
        for r in range(4):
            idx_arr[(key, r)] = _wrap_idx(pr["idx16"][r])

    # ---------- device program ----------
    nc = bacc.Bacc("TRN2", target_bir_lowering=False, debug=False,
                   num_devices=NCORES, num_swdge_queues=4)

    def din(name, shape, dt=bf16):
        return nc.dram_tensor(name, list(shape), dt, kind="ExternalInput").ap()

    xt = din("x_tab", (TABROWS, P))
    w1t = din("W1r", (P, P))
    w2t = din("W2r", (P, P))
    wab = din("Wab", (P, 4))           # [A.T | B.T] columns: [f, 4]
    iota_in = din("iota", (P, P))
    idx_in = {k: din(f"idx_{k[0]}_{k[1]}", v.shape[1:], i16)
              for k, v in idx_arr.items()}
    relw_in = {key: (din(f"rel_{key}", (P, pr["n_sel"]), f32),
                     din(f"w_{key}", (P, pr["n_sel"]), f32))
               for key, pr in (("l1", l1), ("l2", l2))}
    uidx_in = din("u_idx", u_idx16.shape[1:], i16)
    vidx_in = din("v_idx", v_idx16.shape[1:], i16)

    out_u = nc.dram_tensor("out_u", [P, 2 * ndchunks], f32,
                           kind="ExternalOutput").ap()
    out_v = nc.dram_tensor("out_v", [P, 2 * ndchunks], f32,
                           kind="ExternalOutput").ap()
    h_slice = nc.dram_tensor("h_slice", [SLOTS, P], bf16)
    h_tab = nc.dram_tensor("h_tab", [TABROWS, P], bf16, addr_space="Shared")

    qn = [0]

    def next_q():
        qn[0] = (qn[0] + 1) % 4
        return qn[0]

    with tile.TileContext(nc) as tc:
        with (
            tc.tile_pool(name="meta", bufs=1) as mp,
            tc.tile_pool(name="stage", bufs=2) as sgp,
            tc.tile_pool(name="idxp", bufs=1) as ixp,
            tc.tile_pool(name="selp", bufs=4) as selp,
            tc.tile_pool(name="work", bufs=3) as wp,
            tc.tile_pool(name="zg", bufs=2) as zgp,
            tc.tile_pool(name="psA", bufs=2, space="PSUM") as ppA,
            tc.tile_pool(name="psB", bufs=2, space="PSUM") as ppB,
            tc.tile_pool(name="psC", bufs=4, space="PSUM") as ppC,
        ):
            nc.gpsimd.load_library(mlp)
            iota_t = mp.tile([P, P], bf16, name="iota_t")
            nc.sync.dma_start(iota_t[:], iota_in[:])
            w1_sb = mp.tile([P, P], bf16, name="w1_sb")
            nc.sync.dma_start(w1_sb[:], w1t[:])
            w2_sb = mp.tile([P, P], bf16, name="w2_sb")
            nc.sync.dma_start(w2_sb[:], w2t[:])
            wab_sb = mp.tile([P, 4], bf16, name="wab_sb")
            nc.sync.dma_start(wab_sb[:], wab[:])
            z_sb = mp.tile([P, CHUNKS * P], bf16, name="z_sb")

            def sel_build(name, rel_sb, w_sb, col):
                sel = selp.tile([P, P], bf16, name=name, tag="sel")
                nc.vector.tensor_scalar(
                    out=sel[:], in0=iota_t[:],
                    scalar1=rel_sb[:, col:col + 1],
                    scalar2=w_sb[:, col:col + 1],
                    op0=mybir.AluOpType.is_equal,
                    op1=mybir.AluOpType.mult)
                return sel

            def run_agg(key, pr, tab_ap, consume):
                """Gather + sel-route; consume(k, psum[f,d]) per chunk."""
                rel_sb = ixp.tile([P, pr["n_sel"]], f32,
                                  name=f"rel_{key}_sb", tag="relt")
                w_sb = ixp.tile([P, pr["n_sel"]], f32,
                                name=f"w_{key}_sb", tag="wt")
                nc.sync.dma_start(rel_sb[:], relw_in[key][0][:])
                nc.sync.dma_start(w_sb[:], relw_in[key][1][:])
                idx_sb = []
                for r in range(4):
                    cols = pr["layout"][r]["T"] // 16
                    it = ixp.tile([P, cols], i16, name=f"ix_{key}_{r}",
                                  tag=f"ix{r}")
                    nc.sync.dma_start(it[:], idx_in[(key, r)][:])
                    idx_sb.append(it)
                stage_tiles = {}

                def ensure_call(r, ci):
                    if (r, ci) in stage_tiles:
                        return stage_tiles[(r, ci)]
                    call = pr["layout"][r]["calls"][ci]
                    npad = call["n_pad"]
                    c0 = call["base"] // 16
                    st = sgp.tile([P, (npad // P) * P], bf16,
                                  name=f"st_{key}_{r}_{ci}", tag=f"stage{r}")
                    nc.gpsimd.dma_gather(
                        st[:].rearrange("p (c e) -> p c e", e=P),
                        tab_ap[RLO[r]:], idx_sb[r][:, c0:c0 + npad // 16],
                        npad, npad, P,
                        queue_num=next_q(), single_packet=False)
                    stage_tiles[(r, ci)] = st
                    return st

                for k in range(CHUNKS):
                    psum_k = ppA.tile([P, P], f32, space="PSUM",
                                      name=f"ps_{key}_{k}", tag="psA")
                    descs = pr["sched"][k]
                    for j, d in enumerate(descs):
                        st = ensure_call(d["r"], d["call"])
                        sel = sel_build(f"sel_{key}_{k}_{j}", rel_sb, w_sb,
                                        d["sel"])
                        nc.tensor.matmul(
                            psum_k[:],
                            lhsT=st[:, d["blk"] * P:(d["blk"] + 1) * P],
                            rhs=sel[:],
                            start=(j == 0), stop=(j == len(descs) - 1))
                    consume(k, psum_k)

            def consume_l1(k, psum_k):
                agg_sb = wp.tile([P, P], bf16, name=f"a1_{k}", tag="a")
                nc.scalar.activation(agg_sb[:], psum_k[:], AF.Copy)
                h_ps = ppB.tile([P, P], f32, space="PSUM",
                                name=f"h1_{k}", tag="psB")
                nc.tensor.matmul(h_ps[:], lhsT=agg_sb[:], rhs=w1_sb[:],
                                 start=True, stop=True)
                h_sb = wp.tile([P, P], bf16, name=f"h1s_{k}", tag="h")
                nc.scalar.activation(h_sb[:], h_ps[:], AF.Relu)
                nc.sync.dma_start(h_slice[k * P:(k + 1) * P, :], h_sb[:])

            def consume_l2(k, psum_k):
                agg_sb = wp.tile([P, P], bf16, name=f"a2_{k}", tag="a")
                nc.scalar.activation(agg_sb[:], psum_k[:], AF.Copy)
                z_ps = ppB.tile([P, P], f32, space="PSUM",
                                name=f"z2_{k}", tag="psB")
                nc.tensor.matmul(z_ps[:], lhsT=agg_sb[:], rhs=w2_sb[:],
                                 start=True, stop=True)
                nc.scalar.activation(z_sb[:, k * P:(k + 1) * P], z_ps[:],
                                     AF.Copy)

            run_agg("l1", l1, xt, consume_l1)
            nc.gpsimd.collective_compute(
                "AllGather", mybir.AluOpType.bypass,
                replica_groups=[list(range(NCORES))],
                ins=[h_slice[:]], outs=[h_tab[:]])
            run_agg("l2", l2, h_tab[:], consume_l2)

            # ---- decode: SBUF-source transpose gather of z rows ----
            uix_sb = ixp.tile([P, nd_pairs // 16], i16, name="uix", tag="uix")
            nc.sync.dma_start(uix_sb[:], uidx_in[:])
            vix_sb = ixp.tile([P, nd_pairs // 16], i16, name="vix", tag="vix")
            nc.sync.dma_start(vix_sb[:], vidx_in[:])
            outu_sb = mp.tile([P, 2 * ndchunks], f32, name="outu_sb")
            outv_sb = mp.tile([P, 2 * ndchunks], f32, name="outv_sb")

            def run_dec(pass_name, ix_sb, wcol, out_sb):
                for wi, (p0, p1) in enumerate(dec_wins):
                    nwin = p1 - p0
                    zg = zgp.tile([P, nwin], bf16,
                                  name=f"zg_{pass_name}_{wi}", tag="zg")
                    nc.gpsimd.dma_gather(
                        zg[:].rearrange("p (c e) -> p c e", c=1),
                        z_sb[:], ix_sb[:, p0 // 16:p1 // 16],
                        nwin, nwin, P,
                        transpose=True,
                        queue_num=0, single_packet=False,
                        sbuf_tokens_per_rank=P,
                        sbuf_free_dim_per_rank=2 * P,
                    )
                    for k in range(p0 // P, p1 // P):
                        kk = k - p0 // P
                        o_ps = ppC.tile([P, 2], f32, space="PSUM",
                                        name=f"o_{pass_name}_{k}", tag="psC")
                        nc.tensor.matmul(
                            o_ps[:],
                            lhsT=zg[:, kk * P:(kk + 1) * P],
                            rhs=wab_sb[:, wcol:wcol + 2],
                            start=True, stop=True)
                        nc.scalar.activation(out_sb[:, 2 * k:2 * k + 2],
                                             o_ps[:], AF.Copy)

            run_dec("u", uix_sb, 0, outu_sb)
            run_dec("v", vix_sb, 2, outv_sb)
            nc.sync.dma_start(out_u[:], outu_sb[:])
            nc.sync.dma_start(out_v[:], outv_sb[:])

    nc.compile()

    # ---------- stage inputs & run ----------
    iota_np = np.broadcast_to(np.arange(P, dtype=np.float32)[None, :],
                              (P, P)).astype(BF16)
    wab_np = np.ascontiguousarray(
        np.concatenate([Wlin[:, :P].T, Wlin[:, P:].T], axis=1)).astype(BF16)
    in_maps = []
    for c in range(NCORES):
        m = {"x_tab": x_tab, "W1r": W1.astype(BF16),
             "W2r": W2.astype(BF16), "Wab": wab_np, "iota": iota_np,
             "u_idx": u_idx16[c], "v_idx": v_idx16[c]}
        for key, pr in (("l1", l1), ("l2", l2)):
            m[f"rel_{key}"] = np.ascontiguousarray(pr["rel"][c])
            m[f"w_{key}"] = np.ascontiguousarray(pr["wgt"][c])
            for r in range(4):
                m[f"idx_{key}_{r}"] = idx_arr[(key, r)][c]
        in_maps.append(m)

    res = run_bass_kernel_spmd(nc, in_maps, core_ids=list(range(NCORES)),
                               trace=globals().get("TRACE", False))
    globals()["LAST_EXEC_NS"] = res.exec_time_ns

    out = np.zeros((npairs, 2), np.float32)
    for c in range(NCORES):
        u3 = res.results[c]["out_u"].reshape(P, ndchunks, 2)
        nuc = len(u_ids[c])
        sl = np.arange(nuc)
        out[u_ids[c]] += u3[sl % P, sl // P]
        v3 = res.results[c]["out_v"].reshape(P, ndchunks, 2)
        nvc = len(v_ids[c])
        sl = np.arange(nvc)
        out[v_ids[c]] += v3[sl % P, sl // P]
    return out


# revision 9
# speedup vs baseline: 1.4243x; 1.2014x over previous
"""GCN 2-layer + link decode on 8 TRN2 NeuronCores (full inputs in/out).

v3 design (dest-sharded, scatter-free, bf16, host-built sel):
- Aggregation commutes with the weight matmul: h = relu(segsum(w1*x[src]) @ W1);
  z = segsum2(w2*h[src]) @ W2.
- All tables/stages/sels bf16 (fp32 matmul is 2-pass on PE; bf16 is 1).
- Flipped routing matmul: psum[f,d] += stage[e,f].T @ sel[e,d] — the [f,d]
  psum feeds matmul(lhsT=agg[f,d], rhs=W) directly, no transposes.
- sel one-hot*weight strips are built ON HOST and DMA'd per gather window
  (DVE builds measured 507ns/desc and serialized the whole kernel in v2).
- Each core owns 12544 destination slots; edge streams sorted by
  (range, dest chunk, slot); bulk dma_gather with int16 local indices per
  32768-row range; cell sizes max-padded across cores (SPMD-uniform).
- Small gather windows (~1k rows/call) rotate across 4 SWDGE queues so the
  per-queue descriptor rings drain in parallel (desc-gen is ring-throttled).
- h AllGather'd between layers (bf16); z stays RESIDENT IN SBUF.
- Decode: pairs sharded by owner(endpoint) per side, sorted by local z slot;
  host-built sel routes SBUF z chunks into pair-chunk psums [f,pairs];
  per chunk one matmul with the Wlin half gives [pair,2]; host adds u+v.
"""
import numpy as np
import ml_dtypes

BF16 = ml_dtypes.bfloat16
P = 128
N = 100_000
NSHARD = 12_500
SLOTS = 12_544
CHUNKS = SLOTS // P          # 98
TABROWS = 8 * SLOTS          # 100352
RBOUND = [32768, 65536, 98304]
RLO = [0, 32768, 65536, 98304]
NCORES = 8
WINDOW = 4                   # chunks per gather-call window (l1/l2)
DEC_WIN = 32                 # pair-chunks per decode sel-strip window


def _range_of(a):
    return np.searchsorted(RBOUND, a, side="right")


def _wrap_idx(a):
    """[NCORES, T] int16 -> [NCORES, 128, T//16] (16-wrap, 8x replicate)."""
    ncr, t = a.shape
    out = a.reshape(ncr, t // 16, 16).transpose(0, 2, 1)
    return np.ascontiguousarray(np.tile(out, (1, 8, 1)))


def _prep_stream(tab_row, slot, w, nchunks, window):
    """SPMD-uniform gather+route stream builder (dest-major, 4 ranges).

    Returns static schedule + per-core idx16 and host-built sel strips
    sel[c] = [128, n_sel*128] bf16 with sel[pp, col*128+m] = w for the
    entry at block-position pp of desc col routing to chunk-slot m.
    """
    ncr = len(tab_row)
    counts = np.zeros((ncr, nchunks, 4), np.int64)
    for c in range(ncr):
        np.add.at(counts, (c, slot[c] // P, _range_of(tab_row[c])), 1)
    estar = counts.max(axis=0)                       # [nchunks, 4]

    layout = []
    for r in range(4):
        calls = []
        base = 0
        for k0 in range(0, nchunks, window):
            k1 = min(k0 + window, nchunks)
            cells = estar[k0:k1, r]
            offs = np.concatenate([[0], np.cumsum(cells)]).astype(np.int64)
            n = int(offs[-1])
            n_pad = max(P, ((n + P - 1) // P) * P)
            calls.append(dict(k0=k0, k1=k1, offs=offs, n=n, n_pad=n_pad,
                              base=base))
            base += n_pad
        layout.append(dict(calls=calls, T=base))

    sched = [[] for _ in range(nchunks)]
    selmap = {}
    callsel = {}                                     # (r,ci) -> (s0, s1)
    n_sel = 0
    for r in range(4):
        for ci, call in enumerate(layout[r]["calls"]):
            s0 = n_sel
            nblk = call["n_pad"] // P
            offs, k0 = call["offs"], call["k0"]
            for b in range(nblk):
                e0, e1 = b * P, b * P + P
                ks = [k for k in range(call["k0"], call["k1"])
                      if offs[k - k0] < e1 and offs[k - k0 + 1] > e0]
                if not ks:
                    ks = [call["k0"]]
                for k in ks:
                    sched[k].append(dict(r=r, call=ci, blk=b, sel=n_sel))
                    selmap[(r, ci, b, k)] = n_sel
                    n_sel += 1
            callsel[(r, ci)] = (s0, n_sel)

    idx16 = [np.zeros((ncr, layout[r]["T"]), np.int16) for r in range(4)]
    sel = [np.zeros((P, n_sel * P), BF16) for _ in range(ncr)]

    for c in range(ncr):
        tr, sl, ww = tab_row[c], slot[c], w[c]
        rr = _range_of(tr)
        ch = sl // P
        o = np.lexsort((sl, ch, rr))
        tr, sl, ww, rr, ch = tr[o], sl[o], ww[o], rr[o], ch[o]
        for r in range(4):
            m = rr == r
            if not m.any():
                continue
            trm, slm, wwm, chm = tr[m], sl[m], ww[m], ch[m]
            cell_cnt = np.zeros(nchunks, np.int64)
            np.add.at(cell_cnt, chm, 1)
            cstart = np.concatenate([[0], np.cumsum(cell_cnt)])
            within = np.arange(len(slm)) - cstart[chm]
            call_id = chm // window
            calls = layout[r]["calls"]
            cbase = np.array([cl["base"] for cl in calls], np.int64)
            cell_off = np.zeros(nchunks, np.int64)
            for ci, cl in enumerate(calls):
                for k in range(cl["k0"], cl["k1"]):
                    cell_off[k] = cl["offs"][k - cl["k0"]]
            pos = cbase[call_id] + cell_off[chm] + within
            idx16[r][c, pos] = (trm - RLO[r]).astype(np.int16)
            relpos = pos - cbase[call_id]
            blk = relpos // P
            pp = relpos % P
            cols = np.array([selmap[(r, int(ci_), int(b_), int(k_))]
                             for ci_, b_, k_ in zip(call_id, blk, chm)],
                            np.int64)
            sel[c][pp, cols * P + (slm % P)] = wwm
    return dict(layout=layout, sched=sched, n_sel=n_sel, idx16=idx16,
                sel=sel, callsel=callsel)


def _prep_decode(tt):
    """Per-core pair routing with a core-uniform slot layout.

    Pairs are sharded by owner(tt). Slot space reserves cap_j =
    max-over-cores(count of pairs whose z row is in chunk j) slots per
    z-chunk j, so the (pair-chunk k -> z-chunk j) desc schedule is
    identical on every core (SPMD). Returns per-core pair ids + slots,
    the static desc schedule, and host-built sel strips.
    """
    owner = tt // NSHARD
    loc = tt - owner * NSHARD
    ids, locs = [], []
    nj = np.zeros((NCORES, CHUNKS), np.int64)
    for c in range(NCORES):
        sel_ids = np.nonzero(owner == c)[0]
        o = np.argsort(loc[sel_ids], kind="stable")
        ids.append(sel_ids[o])
        locs.append(loc[sel_ids][o])
        np.add.at(nj, (c, locs[c] // P), 1)
    cap = nj.max(axis=0)                              # [CHUNKS]
    offs = np.concatenate([[0], np.cumsum(cap)]).astype(np.int64)
    nd = ((int(offs[-1]) + P - 1) // P) * P
    ndchunks = nd // P
    # static schedule: pair-chunk k needs z-chunk j iff slot ranges overlap
    sched = []                                        # [k] -> [j...]
    selmap = {}
    n_sel = 0
    for k in range(ndchunks):
        js = [j for j in range(CHUNKS)
              if offs[j] < (k + 1) * P and offs[j + 1] > k * P]
        if not js:
            js = [0]
        sched.append(js)
        for j in js:
            selmap[(k, j)] = n_sel
            n_sel += 1
    col0 = np.zeros(ndchunks + 1, np.int64)
    for k in range(ndchunks):
        col0[k + 1] = col0[k] + len(sched[k])
    sel = [np.zeros((P, n_sel * P), BF16) for _ in range(NCORES)]
    slots = []
    for c in range(NCORES):
        rows = locs[c]
        j_of = rows // P
        within = np.arange(len(rows)) - np.concatenate(
            [[0], np.cumsum(nj[c])])[j_of]
        sl = offs[j_of] + within
        slots.append(sl)
        k_of = sl // P
        cols = np.array([selmap[(int(k_), int(j_))]
                         for k_, j_ in zip(k_of, j_of)], np.int64)
        sel[c][rows % P, cols * P + (sl % P)] = 1.0
    return dict(ids=ids, slots=slots, nd=nd, ndchunks=ndchunks,
                sched=sched, selmap=selmap, n_sel=n_sel, sel=sel,
                col0=col0)


def kernel(x, edge_index1, edge_index2, edge_weight1, edge_weight2,
           pos_edge_index, W1, W2, Wlin):
    import concourse.bass as bass
    from concourse import bacc, tile, mybir
    from concourse.bass_utils import run_bass_kernel_spmd
    from concourse.library_config import mlp

    f32, bf16, i16 = mybir.dt.float32, mybir.dt.bfloat16, mybir.dt.int16
    i32 = mybir.dt.int32
    AF = mybir.ActivationFunctionType
    x = np.asarray(x, np.float32)
    W1 = np.asarray(W1, np.float32)
    W2 = np.asarray(W2, np.float32)
    Wlin = np.asarray(Wlin, np.float32)
    e1 = np.asarray(edge_index1).astype(np.int64)
    e2 = np.asarray(edge_index2).astype(np.int64)
    w1 = np.asarray(edge_weight1, np.float32)
    w2 = np.asarray(edge_weight2, np.float32)
    pe = np.asarray(pos_edge_index).astype(np.int64)

    # ---------- host index preprocessing ----------
    x_tab = np.zeros((TABROWS, P), BF16)
    x_tab[:N] = x.astype(BF16)
    n2row = (np.arange(N) // NSHARD) * SLOTS + (np.arange(N) % NSHARD)

    def shard_by_dest(src_rows, dst, w):
        owner = dst // NSHARD
        ld = dst - owner * NSHARD
        return ([src_rows[owner == c] for c in range(NCORES)],
                [ld[owner == c] for c in range(NCORES)],
                [w[owner == c] for c in range(NCORES)])

    l1 = _prep_stream(*shard_by_dest(e1[0], e1[1], w1), CHUNKS, WINDOW)
    l2 = _prep_stream(*shard_by_dest(n2row[e2[0]], e2[1], w2),
                      CHUNKS, WINDOW)
    du = _prep_decode(pe[0])
    dv = _prep_decode(pe[1])
    npairs = pe.shape[1]

    idx_arr = {}
    for key, pr in (("l1", l1), ("l2", l2)):
        for r in range(4):
            idx_arr[(key, r)] = _wrap_idx(pr["idx16"][r])

    # ---------- device program ----------
    nc = bacc.Bacc("TRN2", target_bir_lowering=False, debug=False,
                   num_devices=NCORES, num_swdge_queues=4)

    def din(name, shape, dt=bf16):
        return nc.dram_tensor(name, list(shape), dt, kind="ExternalInput").ap()

    xt = din("x_tab", (TABROWS, P))
    w1t = din("W1r", (P, P))
    w2t = din("W2r", (P, P))
    wab = din("Wab", (P, 4))           # [A.T | B.T] columns: [f, 4]
    idx_in = {k: din(f"idx_{k[0]}_{k[1]}", v.shape[1:], i16)
              for k, v in idx_arr.items()}
    sel_in = {key: din(f"sel_{key}", (P, pr["n_sel"] * P))
              for key, pr in (("l1", l1), ("l2", l2), ("u", du), ("v", dv))}

    out_u = nc.dram_tensor("out_u", [P, 2 * du["ndchunks"]], f32,
                           kind="ExternalOutput").ap()
    out_v = nc.dram_tensor("out_v", [P, 2 * dv["ndchunks"]], f32,
                           kind="ExternalOutput").ap()
    h_slice = nc.dram_tensor("h_slice", [SLOTS, P], bf16)
    h_tab = nc.dram_tensor("h_tab", [TABROWS, P], bf16, addr_space="Shared")

    qn = [0]

    def next_q():
        qn[0] = (qn[0] + 1) % 4
        return qn[0]

    with tile.TileContext(nc) as tc:
        with (
            tc.tile_pool(name="meta", bufs=1) as mp,
            tc.tile_pool(name="stage", bufs=3) as sgp,
            tc.tile_pool(name="idxp", bufs=1) as ixp,
            tc.tile_pool(name="selp", bufs=3) as selp,
            tc.tile_pool(name="work", bufs=3) as wp,
            tc.tile_pool(name="psA", bufs=2, space="PSUM") as ppA,
            tc.tile_pool(name="psB", bufs=2, space="PSUM") as ppB,
            tc.tile_pool(name="psC", bufs=4, space="PSUM") as ppC,
        ):
            nc.gpsimd.load_library(mlp)
            w1_sb = mp.tile([P, P], bf16, name="w1_sb")
            nc.sync.dma_start(w1_sb[:], w1t[:])
            w2_sb = mp.tile([P, P], bf16, name="w2_sb")
            nc.sync.dma_start(w2_sb[:], w2t[:])
            wab_sb = mp.tile([P, 4], bf16, name="wab_sb")
            nc.sync.dma_start(wab_sb[:], wab[:])
            z_sb = mp.tile([P, CHUNKS * P], bf16, name="z_sb")

            def run_agg(key, pr, tab_ap, consume):
                """Gather + sel-route; consume(k, psum[f,d]) per chunk."""
                idx_sb = []
                for r in range(4):
                    cols = pr["layout"][r]["T"] // 16
                    it = ixp.tile([P, cols], i16, name=f"ix_{key}_{r}",
                                  tag=f"ix{r}")
                    nc.sync.dma_start(it[:], idx_in[(key, r)][:])
                    idx_sb.append(it)
                stage_tiles = {}
                sel_tiles = {}

                def ensure_call(r, ci):
                    if (r, ci) in stage_tiles:
                        return
                    call = pr["layout"][r]["calls"][ci]
                    npad = call["n_pad"]
                    c0 = call["base"] // 16
                    st = sgp.tile([P, (npad // P) * P], bf16,
                                  name=f"st_{key}_{r}_{ci}", tag=f"stage{r}")
                    nc.gpsimd.dma_gather(
                        st[:].rearrange("p (c e) -> p c e", e=P),
                        tab_ap[RLO[r]:], idx_sb[r][:, c0:c0 + npad // 16],
                        npad, npad, P,
                        queue_num=next_q(), single_packet=False)
                    s0, s1 = pr["callsel"][(r, ci)]
                    se = selp.tile([P, (s1 - s0) * P], bf16,
                                   name=f"se_{key}_{r}_{ci}", tag=f"sel{r}")
                    nc.sync.dma_start(se[:], sel_in[key][:, s0 * P:s1 * P])
                    stage_tiles[(r, ci)] = st
                    sel_tiles[(r, ci)] = (se, s0)

                for k in range(CHUNKS):
                    psum_k = ppA.tile([P, P], f32, space="PSUM",
                                      name=f"ps_{key}_{k}", tag="psA")
                    descs = pr["sched"][k]
                    for j, d in enumerate(descs):
                        ensure_call(d["r"], d["call"])
                        st = stage_tiles[(d["r"], d["call"])]
                        se, s0 = sel_tiles[(d["r"], d["call"])]
                        sc = d["sel"] - s0
                        nc.tensor.matmul(
                            psum_k[:],
                            lhsT=st[:, d["blk"] * P:(d["blk"] + 1) * P],
                            rhs=se[:, sc * P:(sc + 1) * P],
                            start=(j == 0), stop=(j == len(descs) - 1))
                    consume(k, psum_k)

            def consume_l1(k, psum_k):
                agg_sb = wp.tile([P, P], bf16, name=f"a1_{k}", tag="a")
                nc.scalar.activation(agg_sb[:], psum_k[:], AF.Copy)
                h_ps = ppB.tile([P, P], f32, space="PSUM",
                                name=f"h1_{k}", tag="psB")
                nc.tensor.matmul(h_ps[:], lhsT=agg_sb[:], rhs=w1_sb[:],
                                 start=True, stop=True)
                h_sb = wp.tile([P, P], bf16, name=f"h1s_{k}", tag="h")
                nc.scalar.activation(h_sb[:], h_ps[:], AF.Relu)
                nc.sync.dma_start(h_slice[k * P:(k + 1) * P, :], h_sb[:])

            def consume_l2(k, psum_k):
                agg_sb = wp.tile([P, P], bf16, name=f"a2_{k}", tag="a")
                nc.scalar.activation(agg_sb[:], psum_k[:], AF.Copy)
                z_ps = ppB.tile([P, P], f32, space="PSUM",
                                name=f"z2_{k}", tag="psB")
                nc.tensor.matmul(z_ps[:], lhsT=agg_sb[:], rhs=w2_sb[:],
                                 start=True, stop=True)
                nc.scalar.activation(z_sb[:, k * P:(k + 1) * P], z_ps[:],
                                     AF.Copy)

            run_agg("l1", l1, xt, consume_l1)
            nc.gpsimd.collective_compute(
                "AllGather", mybir.AluOpType.bypass,
                replica_groups=[list(range(NCORES))],
                ins=[h_slice[:]], outs=[h_tab[:]])
            run_agg("l2", l2, h_tab[:], consume_l2)

            # ---- decode: host-sel routing of SBUF z chunks ----
            outu_sb = mp.tile([P, 2 * du["ndchunks"]], f32, name="outu_sb")
            outv_sb = mp.tile([P, 2 * dv["ndchunks"]], f32, name="outv_sb")

            def run_dec(pname, pr, wcol, out_sb):
                nwin = (pr["ndchunks"] + DEC_WIN - 1) // DEC_WIN
                for wi in range(nwin):
                    k0 = wi * DEC_WIN
                    k1 = min(k0 + DEC_WIN, pr["ndchunks"])
                    s0, s1 = int(pr["col0"][k0]), int(pr["col0"][k1])
                    se = selp.tile([P, (s1 - s0) * P], bf16,
                                   name=f"sd_{pname}_{wi}", tag="seld")
                    nc.sync.dma_start(se[:], sel_in[pname][:, s0 * P:s1 * P])
                    for k in range(k0, k1):
                        zr_ps = ppB.tile([P, P], f32, space="PSUM",
                                         name=f"zr_{pname}_{k}", tag="psB")
                        js = pr["sched"][k]
                        for di, j in enumerate(js):
                            col = int(pr["col0"][k]) + di
                            nc.tensor.matmul(
                                zr_ps[:],
                                lhsT=z_sb[:, j * P:(j + 1) * P],
                                rhs=se[:, (col - s0) * P:(col - s0 + 1) * P],
                                start=(di == 0), stop=(di == len(js) - 1))
                        zr_sb = wp.tile([P, P], bf16, name=f"zs_{pname}_{k}",
                                        tag="h")
                        nc.scalar.activation(zr_sb[:], zr_ps[:], AF.Copy)
                        o_ps = ppC.tile([P, 2], f32, space="PSUM",
                                        name=f"o_{pname}_{k}", tag="psC")
                        nc.tensor.matmul(o_ps[:], lhsT=zr_sb[:],
                                         rhs=wab_sb[:, wcol:wcol + 2],
                                         start=True, stop=True)
                        nc.scalar.activation(out_sb[:, 2 * k:2 * k + 2],
                                             o_ps[:], AF.Copy)

            run_dec("u", du, 0, outu_sb)
            run_dec("v", dv, 2, outv_sb)
            nc.sync.dma_start(out_u[:], outu_sb[:])
            nc.sync.dma_start(out_v[:], outv_sb[:])

    nc.compile()

    # ---------- stage inputs & run ----------
    wab_np = np.ascontiguousarray(
        np.concatenate([Wlin[:, :P].T, Wlin[:, P:].T], axis=1)).astype(BF16)
    in_maps = []
    for c in range(NCORES):
        m = {"x_tab": x_tab, "W1r": W1.astype(BF16),
             "W2r": W2.astype(BF16), "Wab": wab_np,
             "sel_l1": l1["sel"][c], "sel_l2": l2["sel"][c],
             "sel_u": du["sel"][c], "sel_v": dv["sel"][c]}
        for key in ("l1", "l2"):
            for r in range(4):
                m[f"idx_{key}_{r}"] = idx_arr[(key, r)][c]
        in_maps.append(m)

    res = run_bass_kernel_spmd(nc, in_maps, core_ids=list(range(NCORES)),
                               trace=globals().get("TRACE", False))
    globals()["LAST_EXEC_NS"] = res.exec_time_ns

    out = np.zeros((npairs, 2), np.float32)
    for pr, nm in ((du, "out_u"), (dv, "out_v")):
        for c in range(NCORES):
            o3 = res.results[c][nm].reshape(P, pr["ndchunks"], 2)
            sl = pr["slots"][c]
            out[pr["ids"][c]] += o3[sl % P, sl // P]
    return out
